# revision 1
# baseline (speedup 1.0000x reference)
"""Trainium2 Bass kernel: causal multi-head attention, tensor-parallel over heads.

Problem: x:(2,2048,2048) f32, wq/wk/wv/wo:(2048,2048) f32 (torch Linear layout,
applied as x @ W.T).  out = MHA_causal(x) @ wo.T, 16 heads x 128 dim.

Sharding: 2 heads per core (8 cores).  Each core computes Q/K/V projections for
its 2 heads, causal attention, and a partial out-projection through its slice
of wo; the host sums the 8 partial outputs (the all-reduce equivalent).

Per-core layouts (all compute in fp16, accumulation in f32 PSUM):
  xT   (D, NTOK)   d-major      : streaming rhs for Q^T/K^T proj, stationary for V
  Q^T  (dh, tok)   per (b,h)    : moving operand of scores
  K^T  (dh, tok)   per (b,h)    : stationary operand of scores
  V    (tok, m)    token-major  : stationary operand of AV
  s_T  (key, q)    scores psum  : softmax denominator via DVE partition-tree
  y^T  (dh, tok)   normalized   : stationary operand of out-proj
"""

import sys

for _p in ("/opt/trn_rl_repo", "/root/.axon_site/_ro/trn_rl_repo"):
    if _p not in sys.path:
        sys.path.append(_p)

from contextlib import ExitStack

import ml_dtypes
import numpy as np

import concourse.bass as bass
import concourse.bacc as bacc
import concourse.mybir as mybir
import concourse.tile as tile
from concourse.bass_utils import run_bass_kernel_spmd

# compute dtype: fp16 (same TensorE speed as bf16, 4x DVE mode, and 3 more
# mantissa bits); PSUM accumulation is always f32
CDT = mybir.dt.float16
F32 = mybir.dt.float32
NPCDT = np.float16
# exp(x - 4*ln2) = exp(x)/16 keeps fp16 softmax denominators well inside range;
# the scale cancels exactly in the normalization
EXP_BIAS = -4.0 * float(np.log(2.0))

N_CORES = 8
B, S, D = 2, 2048, 2048
NH, DH = 16, 128
HPC = NH // N_CORES          # heads per core
ML = HPC * DH                # local head dims per core (256)
SCALE = 1.0 / float(np.sqrt(DH))


def build_nc(b=B, s=S, d=D):
    """Build the per-core Bass graph.  Parameterized so a scaled-down variant
    can run under CoreSim; the shipped kernel always uses the defaults."""
    ntok = b * s
    ndch = d // 128       # contraction chunks for projections
    nech = d // 512       # 512-wide out-proj e chunks
    ttch = s // 512       # 512-token chunks per batch
    nqj = s // 512        # 512-query chunks per (b, h)
    nblk = s // 128       # 128-token blocks per batch

    nc = bacc.Bacc("TRN2", target_bir_lowering=False, debug=False,
                   num_devices=N_CORES)
    xT_e = nc.dram_tensor("xT", [d, ntok], CDT, kind="ExternalInput").ap()
    wqT_e = nc.dram_tensor("wqT", [d, ML], CDT, kind="ExternalInput").ap()
    wkT_e = nc.dram_tensor("wkT", [d, ML], CDT, kind="ExternalInput").ap()
    wvT_e = nc.dram_tensor("wvT", [d, ML], CDT, kind="ExternalInput").ap()
    woT_e = nc.dram_tensor("woT", [ML, d], CDT, kind="ExternalInput").ap()
    out_e = nc.dram_tensor("out", [ntok, d], F32, kind="ExternalOutput").ap()

    with tile.TileContext(nc) as tc, ExitStack() as ctx:
        const = ctx.enter_context(tc.tile_pool(name="const", bufs=1))
        wpool = ctx.enter_context(tc.tile_pool(name="wpool", bufs=1))
        xpool = ctx.enter_context(tc.tile_pool(name="xpool", bufs=4))
        qkv = ctx.enter_context(tc.tile_pool(name="qkv", bufs=1))
        epool = ctx.enter_context(tc.tile_pool(name="epool", bufs=8))
        dpool = ctx.enter_context(tc.tile_pool(name="dpool", bufs=3))
        opool = ctx.enter_context(tc.tile_pool(name="opool", bufs=4))
        # explicit PSUM partitioning (8 banks total): 2 long-lived AV
        # accumulators (throttles attention to 2 blocks in flight), 4 for
        # score/projection groups (deep run-ahead so ACT never starves), 2
        # shared by denominator-broadcast and out-projection churn
        pso = ctx.enter_context(tc.tile_pool(name="pso", bufs=2, space="PSUM"))
        pss = ctx.enter_context(tc.tile_pool(name="pss", bufs=2, space="PSUM"))
        psj = ctx.enter_context(tc.tile_pool(name="psj", bufs=2, space="PSUM"))
        psm = ctx.enter_context(tc.tile_pool(name="psm", bufs=2, space="PSUM"))

        # --- constants -----------------------------------------------------
        # ones first: it gates the PE warm-up burst below
        ones_bf = const.tile([128, 128], CDT, tag="ones", name="ones")
        nc.gpsimd.memset(ones_bf[:], 1.0)

        # PE warm-up: ~3us of dummy matmuls on the ones tile while the first
        # DMAs are still in flight, so the HAM clock gate is already open
        # (2.4 GHz) when real work arrives
        warm = psm.tile([128, 512], F32, tag="m", name="warm")
        for _ in range(28):
            nc.tensor.matmul(warm[:, 0:128], ones_bf[:], ones_bf[:],
                             start=True, stop=True, skip_group_check=True)
        wdump = const.tile([128, 128], F32, tag="wdump", name="wdump")
        nc.scalar.copy(wdump[:], warm[:, 0:128])

        # Diagonal causal mask: key row x attends query col y iff y - x >= 0.
        trimask = const.tile([128, 128], CDT, tag="trimask", name="trimask")
        nc.gpsimd.memset(trimask[:], 1.0)
        nc.gpsimd.affine_select(
            out=trimask[:], in_=trimask[:], compare_op=mybir.AluOpType.is_ge,
            fill=0.0, base=0, pattern=[[1, 128]], channel_multiplier=-1)
        ebias = const.tile([128, 1], F32, tag="ebias", name="ebias")
        nc.gpsimd.memset(ebias[:], EXP_BIAS)

        # --- weights (wq and the first x chunk first: they gate the first
        # matmul; the rest can stream in behind) -----------------------------
        def load_w(ext, name):
            t = wpool.tile([128, ndch * ML], CDT, tag=name)
            nc.sync.dma_start(
                t.rearrange("p (k m) -> p k m", k=ndch),
                ext.rearrange("(k p) m -> p k m", p=128))
            return t

        # first QT group needs wq chunk k and xt chunk k together: issue the
        # pieces interleaved on sync (the sync engine serializes DMA issue at
        # ~1us each); everything not needed immediately issues from the
        # scalar engine's HWDGE queue in parallel
        wq_s = wpool.tile([128, ndch * ML], CDT, tag="wq", name="wq_s")
        xt0 = xpool.tile([128, ndch * 512], CDT, tag="xt", name="xt")
        w4 = ndch // 4
        for piece in range(4):
            k0 = piece * w4
            nc.sync.dma_start(
                wq_s[:, k0 * ML:(k0 + w4) * ML]
                .rearrange("p (k m) -> p k m", k=w4),
                wqT_e[k0 * 128:(k0 + w4) * 128, :]
                .rearrange("(k p) m -> p k m", p=128))
            nc.sync.dma_start(
                xt0[:, k0 * 512:(k0 + w4) * 512]
                .rearrange("p (k t) -> p k t", k=w4),
                xT_e[k0 * 128:(k0 + w4) * 128, 0:512]
                .rearrange("(k p) t -> p k t", p=128))

        def load_w_vec(ext, name):
            t = wpool.tile([128, ndch * ML], CDT, tag=name, name=name)
            nc.scalar.dma_start(
                t.rearrange("p (k m) -> p k m", k=ndch),
                ext.rearrange("(k p) m -> p k m", p=128))
            return t

        wk_s = load_w_vec(wkT_e, "wk")
        wv_s = load_w_vec(wvT_e, "wv")
        wo_s = []
        for h in range(HPC):
            t = wpool.tile([128, d], CDT, tag=f"wo{h}", name=f"wo{h}")
            nc.scalar.dma_start(t[:], woT_e[h * 128:(h + 1) * 128, :])
            wo_s.append(t)

        # --- persistent per-batch activations ------------------------------
        QT = [[qkv.tile([128, s], CDT, tag=f"qt{bb}{h}", name=f"qt{bb}{h}") for h in range(HPC)]
              for bb in range(b)]
        KT = [[qkv.tile([128, s], CDT, tag=f"kt{bb}{h}", name=f"kt{bb}{h}") for h in range(HPC)]
              for bb in range(b)]
        V = [qkv.tile([128, nblk * ML], CDT, tag=f"v{bb}", name=f"v{bb}") for bb in range(b)]
        YT = [[qkv.tile([128, s], CDT, tag=f"yt{bb}{h}", name=f"yt{bb}{h}") for h in range(HPC)]
              for bb in range(b)]

        xt_tiles = {(0, 0): xt0}

        def load_xt(bb):
            # emit the x-chunk DMAs early in the sync stream so they are
            # never head-of-line blocked behind output DMAs that wait on
            # attention results
            for tt in range(ttch):
                if (bb, tt) in xt_tiles:
                    continue
                t0 = bb * s + tt * 512
                t = xpool.tile([128, ndch * 512], CDT, tag="xt", name="xt")
                nc.sync.dma_start(
                    t.rearrange("p (k t) -> p k t", k=ndch),
                    xT_e[:, t0:t0 + 512].rearrange("(k p) t -> p k t",
                                                   p=128))
                xt_tiles[(bb, tt)] = t

        def qk_group(bb, tt, m2, wsb, dst):
            xt = xt_tiles[(bb, tt)]
            pst = psj.tile([128, 512], F32, tag="j", name="pst")
            for k in range(ndch):
                nc.tensor.matmul(
                    pst[:],
                    wsb[:, k * ML + m2 * 128: k * ML + m2 * 128 + 128],
                    xt[:, k * 512:(k + 1) * 512],
                    start=(k == 0), stop=(k == ndch - 1))
            # alternate the psum->sbuf cast between DVE and ACT so woven
            # projection groups don't pile onto the attention-loaded DVE
            if m2 % 2 == 0:
                nc.vector.tensor_copy(
                    dst[bb][m2][:, tt * 512:(tt + 1) * 512], pst[:])
            else:
                nc.scalar.copy(
                    dst[bb][m2][:, tt * 512:(tt + 1) * 512], pst[:])

        def v_group(bb, tt, v4):
            xt = xt_tiles[(bb, tt)]
            pst = psj.tile([128, 512], F32, tag="j", name="pst")
            for k in range(ndch):
                nc.tensor.matmul(
                    pst[:, 0:ML],
                    xt[:, k * 512 + v4 * 128: k * 512 + v4 * 128 + 128],
                    wv_s[:, k * ML:(k + 1) * ML],
                    start=(k == 0), stop=(k == ndch - 1))
            blk = tt * 4 + v4
            nc.scalar.copy(V[bb][:, blk * ML:(blk + 1) * ML], pst[:, 0:ML])

        def proj_units(bb):
            # phase 1 as a list of independently emittable 16-matmul units so
            # the next batch's projection can be woven between attention
            # blocks of the current one (the per-engine schedule is
            # priority-ordered: stalls can only be filled by work that is
            # EMITTED inside the stall region)
            units = []
            for tt in range(ttch):
                for m2 in range(HPC):
                    for wsb, dst in ((wq_s, QT), (wk_s, KT)):
                        units.append(
                            lambda bb=bb, tt=tt, m2=m2, wsb=wsb, dst=dst:
                            qk_group(bb, tt, m2, wsb, dst))
                for v4 in range(4):
                    units.append(
                        lambda bb=bb, tt=tt, v4=v4: v_group(bb, tt, v4))
            return units

        def outproj_block(bb, n):
            # --- phase 3: partial out-projection for one 128-token block ---
            # fine-grained staging (one tile + DMA per 512-wide e chunk) so
            # the 32 MB output stream drains smoothly in the background
            t0 = n * 128
            for e2 in range((nech + 1) // 2):
                # (128,1024) staging: two psum tiles, one copy on ACT and one
                # on DVE, a single coarser output DMA (halves the sync
                # engine's serial DMA-issue load)
                npair = min(2, nech - e2 * 2)
                ost = opool.tile([128, 1024], F32, tag="ost", name="ost")
                for sub in range(npair):
                    ec = e2 * 2 + sub
                    ps_p = psm.tile([128, 512], F32, tag="m", name="ps_p")
                    for h in range(HPC):
                        nc.tensor.matmul(
                            ps_p[:],
                            YT[bb][h][:, t0:t0 + 128],
                            wo_s[h][:, ec * 512:(ec + 1) * 512],
                            start=(h == 0), stop=(h == HPC - 1))
                    if sub == 0:
                        nc.scalar.copy(ost[:, 0:512], ps_p[:])
                    else:
                        nc.vector.tensor_copy(ost[:, 512:1024], ps_p[:])
                nc.sync.dma_start(
                    out_e[bb * s + t0: bb * s + t0 + 128,
                          e2 * 1024:e2 * 1024 + npair * 512],
                    ost[:, 0:npair * 512])

        def drain(fillers, k):
            for _ in range(min(k, len(fillers))):
                fillers.pop(0)()

        def attn_block(bb, qj, fillers=None, defer_to=None,
                       pe_denom=False):
            # --- phase 2: causal attention for one 512-query block ---------
            # pe_denom: accumulate the softmax denominator with per-ki
            # ones(128,128) matmuls into the (idle) projection PSUM pool
            # instead of DVE adds -- used in the last batch's window where
            # the PE has slack and DVE is the pacer
            for h in range(HPC):
                nki = 4 * qj + 4
                q0 = qj * 512
                ps_o = pso.tile([128, 512], F32, tag="o", name="ps_o")
                if pe_denom:
                    ps_den = psj.tile([128, 512], F32, tag="j",
                                      name="ps_den")
                    acc = acc1 = None
                else:
                    # two independent DVE accumulator chains halve the
                    # serial-add latency per block
                    acc = dpool.tile([128, 512], F32, tag="acc", name="acc")
                    acc1 = dpool.tile([128, 512], F32, tag="acc1",
                                      name="acc1")
                for ki in range(nki):
                    # diagonal key chunks only see queries >= their own
                    # position: narrow to columns [c0:512)
                    g = ki - 4 * qj
                    c0 = 128 * g if g > 0 else 0
                    ps_s = pss.tile([128, 512], F32, tag="s", name="ps_s")
                    nc.tensor.matmul(
                        ps_s[:, c0:512],
                        KT[bb][h][:, ki * 128:(ki + 1) * 128],
                        QT[bb][h][:, q0 + c0:q0 + 512],
                        start=True, stop=True)
                    et = epool.tile([128, 512], CDT, tag="et", name="et")
                    nc.scalar.activation(
                        et[:, c0:512], ps_s[:, c0:512],
                        mybir.ActivationFunctionType.Exp, scale=SCALE,
                        bias=ebias[:, 0:1])
                    if g >= 0:
                        nc.vector.tensor_mul(
                            et[:, c0:c0 + 128], et[:, c0:c0 + 128],
                            trimask[:])
                    if pe_denom:
                        nc.tensor.matmul(
                            ps_den[:, c0:512], ones_bf[:], et[:, c0:512],
                            start=(ki == 0), stop=(ki == nki - 1),
                            skip_group_check=True)
                    else:
                        eng = nc.vector
                        if ki < 2:
                            dst = acc if ki == 0 else acc1
                            if c0 > 0:
                                eng.memset(dst[:, 0:c0], 0.0)
                            eng.tensor_copy(dst[:, c0:512], et[:, c0:512])
                        else:
                            dst = acc if ki % 2 == 0 else acc1
                            eng.tensor_add(dst[:, c0:512], dst[:, c0:512],
                                           et[:, c0:512])
                    nc.tensor.matmul(
                        ps_o[:, c0:512],
                        V[bb][:, ki * ML + h * 128: ki * ML + h * 128 + 128],
                        et[:, c0:512],
                        start=(ki == 0), stop=(ki == nki - 1),
                        skip_group_check=True)
                    if fillers and ki % 3 == 2:
                        drain(fillers, 1)
                rbc = dpool.tile([128, 512], F32, tag="rbc", name="rbc")
                if pe_denom:
                    nc.vector.reciprocal_approx_fast(out=rbc[:],
                                                     in_=ps_den[:])
                else:
                    # one ones(128,128) matmul both sums the 128 partitions
                    # of acc and broadcasts the denominator across all 128
                    # output partitions.
                    accb = dpool.tile([128, 512], CDT, tag="accb",
                                      name="accb")
                    nc.vector.tensor_tensor(accb[:], acc[:], acc1[:],
                                            mybir.AluOpType.add)
                    ps_r = psm.tile([128, 512], F32, tag="m", name="ps_r")
                    nc.tensor.matmul(ps_r[:], ones_bf[:],
                                     accb[:], start=True, stop=True)
                    nc.vector.reciprocal_approx_fast(out=rbc[:],
                                                     in_=ps_r[:])
                nc.vector.tensor_mul(YT[bb][h][:, q0:q0 + 512],
                                     ps_o[:], rbc[:])
            for n in range(4 * qj, 4 * qj + 4):
                if defer_to is not None:
                    defer_to.append(lambda bb=bb, n=n: outproj_block(bb, n))
                else:
                    if fillers:
                        drain(fillers, 1)
                    outproj_block(bb, n)

        # per-batch stagger: the next batch's projection units are emitted
        # INSIDE the current batch's attention blocks (the per-engine
        # schedule is priority-ordered, so softmax-latency stalls can only
        # be filled by work emitted within them).  qj descending: longest
        # attention blocks first.
        deferred = []
        for bb in range(b):
            load_xt(bb)
            if bb == 0:
                for u in proj_units(0):
                    u()
            # fillers for this batch's attention window: the previous
            # batch's deferred out-projection plus the next batch's
            # projection units
            fillers = list(deferred)
            deferred = []
            if bb + 1 < b:
                load_xt(bb + 1)
                fillers += proj_units(bb + 1)
                defer_to = deferred   # push own out-proj into next window
            else:
                defer_to = None
            if bb + 1 < b:
                order = list(range(nqj - 1, -1, -1))
            else:
                # last batch: qj0 first (ready after the first projection
                # chunk, warms the softmax chain early), then longest-first
                order = [0] + list(range(nqj - 1, 0, -1))
            for qj in order:
                attn_block(bb, qj, fillers, defer_to,
                           pe_denom=(bb + 1 == b))
            for u in fillers:
                u()

    nc.compile()
    return nc


_NC_CACHE = {}


def _get_nc():
    if "nc" not in _NC_CACHE:
        _NC_CACHE["nc"] = build_nc()
    return _NC_CACHE["nc"]


def shard_inputs(x, wq, wk, wv, wo):
    """Host-side sharding: 2 heads (256 out dims) per core; fp16 cast."""
    ntok = x.shape[0] * x.shape[1]
    xT = np.ascontiguousarray(
        np.asarray(x, dtype=np.float32).reshape(ntok, D).T).astype(NPCDT)
    in_maps = []
    for c in range(N_CORES):
        sl = slice(c * ML, (c + 1) * ML)
        in_maps.append({
            "xT": xT,
            "wqT": np.ascontiguousarray(np.asarray(wq)[sl].T).astype(NPCDT),
            "wkT": np.ascontiguousarray(np.asarray(wk)[sl].T).astype(NPCDT),
            "wvT": np.ascontiguousarray(np.asarray(wv)[sl].T).astype(NPCDT),
            "woT": np.ascontiguousarray(np.asarray(wo)[:, sl].T).astype(NPCDT),
        })
    return in_maps


def run(inputs, trace=False, trace_cores=None):
    nc = _get_nc()
    in_maps = shard_inputs(inputs["x"], inputs["wq"], inputs["wk"],
                           inputs["wv"], inputs["wo"])
    res = run_bass_kernel_spmd(nc, in_maps, core_ids=list(range(N_CORES)),
                               trace=trace, trace_cores=trace_cores)
    out = res.results[0]["out"].astype(np.float32)
    for c in range(1, N_CORES):
        out = out + res.results[c]["out"]
    return out.reshape(B, S, D), res


def kernel(**inputs) -> np.ndarray:
    out, _ = run(inputs, trace=False)
    return out



# revision 2
# speedup vs baseline: 1.0251x; 1.0251x over previous
"""Trainium2 Bass kernel: causal multi-head attention, tensor-parallel over heads.

Problem: x:(2,2048,2048) f32, wq/wk/wv/wo:(2048,2048) f32 (torch Linear layout,
applied as x @ W.T).  out = MHA_causal(x) @ wo.T, 16 heads x 128 dim.

Sharding: 2 heads per core (8 cores).  Each core computes Q/K/V projections for
its 2 heads, causal attention, and a partial out-projection through its slice
of wo; the host sums the 8 fp16 partial outputs (the all-reduce equivalent).

Per-core layouts (all compute in fp16, accumulation in f32 PSUM):
  xT   (D, NTOK)   d-major      : streaming rhs for Q^T/K^T proj, stationary for V
  Q^T  (dh, tok)   per (b,h)    : moving operand of scores
  K^T  (dh, tok)   per (b,h)    : stationary operand of scores
  V    (tok, m)    token-major  : stationary operand of AV
  s_T  (key, q)    scores psum  : softmax denominator via DVE fp16 chains
  y^T  (dh, tok)   normalized   : stationary operand of out-proj

All HBM->SBUF inputs are pre-packed on the host into the exact on-chip tile
layout so each DMA moves 8-16KB contiguous runs per partition (full rate);
the naive (D, ntok) gather runs at ~78GB/s due to 1KB descriptors.
"""

import sys

for _p in ("/opt/trn_rl_repo", "/root/.axon_site/_ro/trn_rl_repo"):
    if _p not in sys.path:
        sys.path.append(_p)

from contextlib import ExitStack

import ml_dtypes
import numpy as np

import concourse.bass as bass
import concourse.bacc as bacc
import concourse.mybir as mybir
import concourse.tile as tile
from concourse.bass_utils import run_bass_kernel_spmd

# compute dtype: fp16 (same TensorE speed as bf16, 2x DVE mode, and 3 more
# mantissa bits); PSUM accumulation is always f32
CDT = mybir.dt.float16
F32 = mybir.dt.float32
NPCDT = np.float16
# exp(x - 4*ln2) = exp(x)/16 keeps fp16 softmax denominators well inside range;
# the scale cancels exactly in the normalization
EXP_BIAS = -4.0 * float(np.log(2.0))

N_CORES = 8
B, S, D = 2, 2048, 2048
NH, DH = 16, 128
HPC = NH // N_CORES          # heads per core
ML = HPC * DH                # local head dims per core (256)
SCALE = 1.0 / float(np.sqrt(DH))


def build_nc(b=B, s=S, d=D):
    """Build the per-core Bass graph.  Parameterized so a scaled-down variant
    can run under CoreSim; the shipped kernel always uses the defaults."""
    ntok = b * s
    ndch = d // 128       # contraction chunks for projections
    nech = d // 512       # 512-wide out-proj e chunks
    ttch = s // 512       # 512-token chunks per batch
    nqj = s // 512        # 512-query chunks per (b, h)
    nblk = s // 128       # 128-token blocks per batch

    nc = bacc.Bacc("TRN2", target_bir_lowering=False, debug=False,
                   num_devices=N_CORES)
    # host-tiled inputs: xT as per-(bb,tt) tiles of (128, ndch*512) laid out
    # contiguously; w{q,k,v} as (128, ndch*ML) in (p, k, m) order
    xT_e = nc.dram_tensor("xT", [128, b * ttch * ndch * 512], CDT,
                          kind="ExternalInput").ap()
    wqT_e = nc.dram_tensor("wqT", [128, ndch * ML], CDT,
                           kind="ExternalInput").ap()
    wkT_e = nc.dram_tensor("wkT", [128, ndch * ML], CDT,
                           kind="ExternalInput").ap()
    wvT_e = nc.dram_tensor("wvT", [128, ndch * ML], CDT,
                           kind="ExternalInput").ap()
    woT_e = nc.dram_tensor("woT", [ML, d], CDT, kind="ExternalInput").ap()
    out_e = nc.dram_tensor("out", [ntok, d], CDT, kind="ExternalOutput").ap()

    with tile.TileContext(nc) as tc, ExitStack() as ctx:
        const = ctx.enter_context(tc.tile_pool(name="const", bufs=1))
        wpool = ctx.enter_context(tc.tile_pool(name="wpool", bufs=1))
        xpool = ctx.enter_context(tc.tile_pool(name="xpool", bufs=4))
        qkv = ctx.enter_context(tc.tile_pool(name="qkv", bufs=1))
        epool = ctx.enter_context(tc.tile_pool(name="epool", bufs=8))
        dpool = ctx.enter_context(tc.tile_pool(name="dpool", bufs=3))
        opool = ctx.enter_context(tc.tile_pool(name="opool", bufs=4))
        # explicit PSUM partitioning (8 banks total): 2 long-lived AV
        # accumulators (throttles attention to 2 blocks in flight), 2 for
        # score groups, 2 for projection groups, 2 shared by the
        # denominator-broadcast and out-projection churn; out-proj also
        # borrows psj (free during attention windows) for 4-bank pipelining
        pso = ctx.enter_context(tc.tile_pool(name="pso", bufs=2, space="PSUM"))
        pss = ctx.enter_context(tc.tile_pool(name="pss", bufs=2, space="PSUM"))
        psj = ctx.enter_context(tc.tile_pool(name="psj", bufs=2, space="PSUM"))
        psm = ctx.enter_context(tc.tile_pool(name="psm", bufs=2, space="PSUM"))

        # --- constants -----------------------------------------------------
        # ones first: it gates the PE warm-up burst below
        ones_bf = const.tile([128, 128], CDT, tag="ones", name="ones")
        nc.gpsimd.memset(ones_bf[:], 1.0)

        # PE warm-up: ~2us of dummy matmuls on the ones tile while the first
        # DMAs are still in flight, so the HAM clock gate is already open
        # when real work arrives
        warm = psm.tile([128, 512], F32, tag="m", name="warm")
        for _ in range(16):
            nc.tensor.matmul(warm[:, 0:128], ones_bf[:], ones_bf[:],
                             start=True, stop=True, skip_group_check=True)
        wdump = const.tile([128, 128], F32, tag="wdump", name="wdump")
        nc.scalar.copy(wdump[:], warm[:, 0:128])

        # Diagonal causal mask: key row x attends query col y iff y - x >= 0.
        trimask = const.tile([128, 128], CDT, tag="trimask", name="trimask")
        nc.gpsimd.memset(trimask[:], 1.0)
        nc.gpsimd.affine_select(
            out=trimask[:], in_=trimask[:], compare_op=mybir.AluOpType.is_ge,
            fill=0.0, base=0, pattern=[[1, 128]], channel_multiplier=-1)
        ebias = const.tile([128, 1], F32, tag="ebias", name="ebias")
        nc.gpsimd.memset(ebias[:], EXP_BIAS)

        # --- weights (wq and the first x chunk first: they gate the first
        # matmul; the rest can stream in behind) -----------------------------
        # first QT group needs wq chunk k and xt chunk k together: issue the
        # pieces interleaved on sync in 4-chunk granules so the k-loop can
        # start as soon as the first granule lands
        wq_s = wpool.tile([128, ndch * ML], CDT, tag="wq", name="wq_s")
        xt0 = xpool.tile([128, ndch * 512], CDT, tag="xt", name="xt")
        w4 = ndch // 4
        for piece in range(4):
            k0 = piece * w4
            nc.sync.dma_start(
                wq_s[:, k0 * ML:(k0 + w4) * ML],
                wqT_e[:, k0 * ML:(k0 + w4) * ML])
            nc.sync.dma_start(
                xt0[:, k0 * 512:(k0 + w4) * 512],
                xT_e[:, k0 * 512:(k0 + w4) * 512])

        def load_w_vec(ext, name):
            t = wpool.tile([128, ndch * ML], CDT, tag=name, name=name)
            nc.scalar.dma_start(t[:], ext[:])
            return t

        wk_s = load_w_vec(wkT_e, "wk")
        wv_s = load_w_vec(wvT_e, "wv")
        wo_s = []
        for h in range(HPC):
            t = wpool.tile([128, d], CDT, tag=f"wo{h}", name=f"wo{h}")
            nc.scalar.dma_start(t[:], woT_e[h * 128:(h + 1) * 128, :])
            wo_s.append(t)

        # --- persistent per-batch activations ------------------------------
        QT = [[qkv.tile([128, s], CDT, tag=f"qt{bb}{h}", name=f"qt{bb}{h}") for h in range(HPC)]
              for bb in range(b)]
        KT = [[qkv.tile([128, s], CDT, tag=f"kt{bb}{h}", name=f"kt{bb}{h}") for h in range(HPC)]
              for bb in range(b)]
        V = [qkv.tile([128, nblk * ML], CDT, tag=f"v{bb}", name=f"v{bb}") for bb in range(b)]
        YT = [[qkv.tile([128, s], CDT, tag=f"yt{bb}{h}", name=f"yt{bb}{h}") for h in range(HPC)]
              for bb in range(b)]

        xt_tiles = {(0, 0): xt0}

        def load_xt(bb):
            # emit the x-chunk DMAs early in the sync stream so they are
            # never head-of-line blocked behind output DMAs that wait on
            # attention results
            for tt in range(ttch):
                if (bb, tt) in xt_tiles:
                    continue
                it = bb * ttch + tt
                t = xpool.tile([128, ndch * 512], CDT, tag="xt", name="xt")
                nc.sync.dma_start(
                    t[:], xT_e[:, it * ndch * 512:(it + 1) * ndch * 512])
                xt_tiles[(bb, tt)] = t

        def qk_group(bb, tt, m2, wsb, dst):
            xt = xt_tiles[(bb, tt)]
            pst = psj.tile([128, 512], F32, tag="j", name="pst")
            for k in range(ndch):
                nc.tensor.matmul(
                    pst[:],
                    wsb[:, k * ML + m2 * 128: k * ML + m2 * 128 + 128],
                    xt[:, k * 512:(k + 1) * 512],
                    start=(k == 0), stop=(k == ndch - 1))
            # alternate the psum->sbuf cast between DVE and ACT so woven
            # projection groups don't pile onto the attention-loaded DVE
            if m2 % 2 == 0:
                nc.vector.tensor_copy(
                    dst[bb][m2][:, tt * 512:(tt + 1) * 512], pst[:])
            else:
                nc.scalar.copy(
                    dst[bb][m2][:, tt * 512:(tt + 1) * 512], pst[:])

        def v_group(bb, tt, v4):
            xt = xt_tiles[(bb, tt)]
            pst = psj.tile([128, 512], F32, tag="j", name="pst")
            for k in range(ndch):
                nc.tensor.matmul(
                    pst[:, 0:ML],
                    xt[:, k * 512 + v4 * 128: k * 512 + v4 * 128 + 128],
                    wv_s[:, k * ML:(k + 1) * ML],
                    start=(k == 0), stop=(k == ndch - 1))
            blk = tt * 4 + v4
            nc.scalar.copy(V[bb][:, blk * ML:(blk + 1) * ML], pst[:, 0:ML])

        def proj_units(bb):
            # phase 1 as a list of independently emittable 16-matmul units so
            # the next batch's projection can be woven between attention
            # blocks of the current one (the per-engine schedule is
            # priority-ordered: stalls can only be filled by work that is
            # EMITTED inside the stall region)
            units = []
            for tt in range(ttch):
                for m2 in range(HPC):
                    for wsb, dst in ((wq_s, QT), (wk_s, KT)):
                        units.append(
                            lambda bb=bb, tt=tt, m2=m2, wsb=wsb, dst=dst:
                            qk_group(bb, tt, m2, wsb, dst))
                for v4 in range(4):
                    units.append(
                        lambda bb=bb, tt=tt, v4=v4: v_group(bb, tt, v4))
            return units

        op_ctr = [0]

        def outproj_block(bb, n):
            # --- phase 3: partial out-projection for one 128-token block ---
            # fine-grained staging (one tile + DMA per 512-wide e chunk) so
            # the output stream drains smoothly in the background; fp16
            # staging halves the HBM write traffic
            t0 = n * 128
            for e2 in range((nech + 1) // 2):
                # (128,1024) staging: two psum tiles, one copy on ACT and one
                # on DVE, a single coarser output DMA (halves the sync
                # engine's serial DMA-issue load).  ps_p alternates between
                # the psm and psj pools: 4 banks of pipelining so the copy
                # latency never stalls the matmul stream (psj is idle during
                # attention windows, which is when out-proj runs)
                npair = min(2, nech - e2 * 2)
                ost = opool.tile([128, 1024], CDT, tag="ost", name="ost")
                for sub in range(npair):
                    ec = e2 * 2 + sub
                    pool = psm if op_ctr[0] % 2 == 0 else psj
                    op_ctr[0] += 1
                    ps_p = pool.tile([128, 512], F32,
                                     tag="m" if pool is psm else "j",
                                     name="ps_p")
                    for h in range(HPC):
                        nc.tensor.matmul(
                            ps_p[:],
                            YT[bb][h][:, t0:t0 + 128],
                            wo_s[h][:, ec * 512:(ec + 1) * 512],
                            start=(h == 0), stop=(h == HPC - 1))
                    if sub == 0:
                        nc.scalar.copy(ost[:, 0:512], ps_p[:])
                    else:
                        nc.vector.tensor_copy(ost[:, 512:1024], ps_p[:])
                nc.sync.dma_start(
                    out_e[bb * s + t0: bb * s + t0 + 128,
                          e2 * 1024:e2 * 1024 + npair * 512],
                    ost[:, 0:npair * 512])

        def drain(fillers, k):
            for _ in range(min(k, len(fillers))):
                fillers.pop(0)()

        def attn_block(bb, qj, fillers=None, defer_to=None):
            # --- phase 2: causal attention for one 512-query block ---------
            # softmax denominator via two fp16 DVE accumulator chains (2x DVE
            # mode; exp values <= ~9.4 and <= 16 summands per element keep
            # fp16 accumulation within ~2e-3 relative)
            for h in range(HPC):
                nki = 4 * qj + 4
                q0 = qj * 512
                ps_o = pso.tile([128, 512], F32, tag="o", name="ps_o")
                # two independent DVE accumulator chains halve the
                # serial-add latency per block
                acc = dpool.tile([128, 512], CDT, tag="acc", name="acc")
                acc1 = dpool.tile([128, 512], CDT, tag="acc1", name="acc1")
                for ki in range(nki):
                    # diagonal key chunks only see queries >= their own
                    # position: narrow to columns [c0:512)
                    g = ki - 4 * qj
                    c0 = 128 * g if g > 0 else 0
                    ps_s = pss.tile([128, 512], F32, tag="s", name="ps_s")
                    nc.tensor.matmul(
                        ps_s[:, c0:512],
                        KT[bb][h][:, ki * 128:(ki + 1) * 128],
                        QT[bb][h][:, q0 + c0:q0 + 512],
                        start=True, stop=True)
                    et = epool.tile([128, 512], CDT, tag="et", name="et")
                    nc.scalar.activation(
                        et[:, c0:512], ps_s[:, c0:512],
                        mybir.ActivationFunctionType.Exp, scale=SCALE,
                        bias=ebias[:, 0:1])
                    if g >= 0:
                        nc.vector.tensor_mul(
                            et[:, c0:c0 + 128], et[:, c0:c0 + 128],
                            trimask[:])
                    eng = nc.vector
                    if ki < 2:
                        dst = acc if ki == 0 else acc1
                        if c0 > 0:
                            eng.memset(dst[:, 0:c0], 0.0)
                        eng.tensor_copy(dst[:, c0:512], et[:, c0:512])
                    else:
                        dst = acc if ki % 2 == 0 else acc1
                        eng.tensor_add(dst[:, c0:512], dst[:, c0:512],
                                       et[:, c0:512])
                    nc.tensor.matmul(
                        ps_o[:, c0:512],
                        V[bb][:, ki * ML + h * 128: ki * ML + h * 128 + 128],
                        et[:, c0:512],
                        start=(ki == 0), stop=(ki == nki - 1),
                        skip_group_check=True)
                    if fillers and ki % 3 == 2:
                        drain(fillers, 1)
                rbc = dpool.tile([128, 512], F32, tag="rbc", name="rbc")
                # one ones(128,128) matmul both sums the 128 partitions
                # of acc and broadcasts the denominator across all 128
                # output partitions.
                accb = dpool.tile([128, 512], CDT, tag="accb", name="accb")
                nc.vector.tensor_tensor(accb[:], acc[:], acc1[:],
                                        mybir.AluOpType.add)
                ps_r = psm.tile([128, 512], F32, tag="m", name="ps_r")
                nc.tensor.matmul(ps_r[:], ones_bf[:],
                                 accb[:], start=True, stop=True)
                nc.vector.reciprocal_approx_fast(out=rbc[:], in_=ps_r[:])
                nc.vector.tensor_mul(YT[bb][h][:, q0:q0 + 512],
                                     ps_o[:], rbc[:])
            for n in range(4 * qj, 4 * qj + 4):
                if defer_to is not None:
                    defer_to.append(lambda bb=bb, n=n: outproj_block(bb, n))
                else:
                    if fillers:
                        drain(fillers, 1)
                    outproj_block(bb, n)

        # per-batch stagger: the next batch's projection units are emitted
        # INSIDE the current batch's attention blocks (the per-engine
        # schedule is priority-ordered, so softmax-latency stalls can only
        # be filled by work emitted within them).  qj descending: longest
        # attention blocks first.
        deferred = []
        for bb in range(b):
            load_xt(bb)
            if bb == 0:
                for u in proj_units(0):
                    u()
            # fillers for this batch's attention window: the previous
            # batch's deferred out-projection plus the next batch's
            # projection units
            fillers = list(deferred)
            deferred = []
            if bb + 1 < b:
                load_xt(bb + 1)
                fillers += proj_units(bb + 1)
                defer_to = deferred   # push own out-proj into next window
            else:
                defer_to = None
            if bb + 1 < b:
                order = list(range(nqj - 1, -1, -1))
            else:
                # last batch: qj0 first (ready after the first projection
                # chunk, warms the softmax chain early), then longest-first
                order = [0] + list(range(nqj - 1, 0, -1))
            for qj in order:
                attn_block(bb, qj, fillers, defer_to)
            for u in fillers:
                u()

    nc.compile()
    return nc


_NC_CACHE = {}


def _get_nc():
    if "nc" not in _NC_CACHE:
        _NC_CACHE["nc"] = build_nc()
    return _NC_CACHE["nc"]


def shard_inputs(x, wq, wk, wv, wo):
    """Host-side sharding: 2 heads (256 out dims) per core; fp16 cast.

    All inputs are packed into the exact on-chip tile layouts so every DMA
    reads long contiguous runs per partition:
      xT  -> (128, b*ttch*ndch*512): tile (bb,tt) at flat index it holds
             [p, it, k, tau] = x[bb, tt*512+tau, k*128+p]
      w?T -> (128, ndch*ML): [p, k, m] = w[core_slice(m), k*128+p]
    """
    ndch = D // 128
    ttch = S // 512
    xf = np.asarray(x, dtype=np.float32).reshape(B, ttch, 512, ndch, 128)
    # [bb, tt, tau, k, p] -> [p, bb, tt, k, tau]
    xT = np.ascontiguousarray(xf.transpose(4, 0, 1, 3, 2)).reshape(
        128, B * ttch * ndch * 512).astype(NPCDT)

    def packw(w, sl):
        # w[sl] is (ML, D); want [p, k, m] = w[sl][m, k*128+p]
        a = np.asarray(w, dtype=np.float32)[sl].reshape(ML, ndch, 128)
        return np.ascontiguousarray(a.transpose(2, 1, 0)).reshape(
            128, ndch * ML).astype(NPCDT)

    in_maps = []
    for c in range(N_CORES):
        sl = slice(c * ML, (c + 1) * ML)
        in_maps.append({
            "xT": xT,
            "wqT": packw(wq, sl),
            "wkT": packw(wk, sl),
            "wvT": packw(wv, sl),
            "woT": np.ascontiguousarray(np.asarray(wo)[:, sl].T).astype(NPCDT),
        })
    return in_maps


def run(inputs, trace=False, trace_cores=None):
    nc = _get_nc()
    in_maps = shard_inputs(inputs["x"], inputs["wq"], inputs["wk"],
                           inputs["wv"], inputs["wo"])
    res = run_bass_kernel_spmd(nc, in_maps, core_ids=list(range(N_CORES)),
                               trace=trace, trace_cores=trace_cores)
    out = res.results[0]["out"].astype(np.float32)
    for c in range(1, N_CORES):
        out = out + res.results[c]["out"].astype(np.float32)
    return out.reshape(B, S, D), res


def kernel(**inputs) -> np.ndarray:
    out, _ = run(inputs, trace=False)
    return out


# revision 5
# speedup vs baseline: 1.1490x; 1.1209x over previous
"""Trainium2 Bass kernel: causal multi-head attention, tensor-parallel over heads.

Problem: x:(2,2048,2048) f32, wq/wk/wv/wo:(2048,2048) f32 (torch Linear layout,
applied as x @ W.T).  out = MHA_causal(x) @ wo.T, 16 heads x 128 dim.

Sharding: 2 heads per core (8 cores).  Each core computes Q/K/V projections for
its 2 heads, causal attention, and a partial out-projection through its slice
of wo; the host sums the 8 fp16 partial outputs (the all-reduce equivalent).

Per-core layouts (all compute in fp16, accumulation in f32 PSUM):
  xT   (D, NTOK)   d-major      : streaming rhs for Q^T/K^T proj, stationary for V
  Q^T  (dh, tok)   per (b,h)    : moving operand of scores
  K^T  (dh, tok)   per (b,h)    : stationary operand of scores
  V    (tok, m)    token-major  : stationary operand of AV
  s_T  (key, q)    scores psum  : softmax denominator via DVE fp16 chains
  y^T  (dh, tok)   normalized   : stationary operand of out-proj

All HBM->SBUF inputs are pre-packed on the host into the exact on-chip tile
layout so each DMA moves 8-16KB contiguous runs per partition (full rate);
the naive (D, ntok) gather runs at ~78GB/s due to 1KB descriptors.
"""

import sys

for _p in ("/opt/trn_rl_repo", "/root/.axon_site/_ro/trn_rl_repo"):
    if _p not in sys.path:
        sys.path.append(_p)

from contextlib import ExitStack

import ml_dtypes
import numpy as np

import concourse.bass as bass
import concourse.bacc as bacc
import concourse.mybir as mybir
import concourse.tile as tile
from concourse.bass_utils import run_bass_kernel_spmd

# compute dtype: fp16 (same TensorE speed as bf16, 2x DVE mode, and 3 more
# mantissa bits); PSUM accumulation is always f32
CDT = mybir.dt.float16
F32 = mybir.dt.float32
NPCDT = np.float16
# exp(x - 4*ln2) = exp(x)/16 keeps fp16 softmax denominators well inside range;
# the scale cancels exactly in the normalization
EXP_BIAS = -4.0 * float(np.log(2.0))

N_CORES = 8
B, S, D = 2, 2048, 2048
NH, DH = 16, 128
HPC = NH // N_CORES          # heads per core
ML = HPC * DH                # local head dims per core (256)
SCALE = 1.0 / float(np.sqrt(DH))


def build_nc(b=B, s=S, d=D):
    """Build the per-core Bass graph.  Parameterized so a scaled-down variant
    can run under CoreSim; the shipped kernel always uses the defaults."""
    ntok = b * s
    ndch = d // 128       # contraction chunks for projections
    nech = d // 512       # 512-wide out-proj e chunks
    ttch = s // 512       # 512-token chunks per batch
    nqj = s // 512        # 512-query chunks per (b, h)
    nblk = s // 128       # 128-token blocks per batch

    nc = bacc.Bacc("TRN2", target_bir_lowering=False, debug=False,
                   num_devices=N_CORES)
    # host-tiled inputs: xT as per-(bb,tt) tiles of (128, ndch*512) laid out
    # contiguously; w{q,k,v} as (128, ndch*ML) in (p, k, m) order
    xT_e = nc.dram_tensor("xT", [128, b * ttch * ndch * 512], CDT,
                          kind="ExternalInput").ap()
    wqT_e = nc.dram_tensor("wqT", [128, ndch * ML], CDT,
                           kind="ExternalInput").ap()
    wkT_e = nc.dram_tensor("wkT", [128, ndch * ML], CDT,
                           kind="ExternalInput").ap()
    wvT_e = nc.dram_tensor("wvT", [128, ndch * ML], CDT,
                           kind="ExternalInput").ap()
    woT_e = nc.dram_tensor("woT", [ML, d], CDT, kind="ExternalInput").ap()
    out_e = nc.dram_tensor("out", [ntok, d], CDT, kind="ExternalOutput").ap()

    with tile.TileContext(nc) as tc, ExitStack() as ctx:
        const = ctx.enter_context(tc.tile_pool(name="const", bufs=1))
        wpool = ctx.enter_context(tc.tile_pool(name="wpool", bufs=1))
        xpool = ctx.enter_context(tc.tile_pool(name="xpool", bufs=4))
        qkv = ctx.enter_context(tc.tile_pool(name="qkv", bufs=1))
        epool = ctx.enter_context(tc.tile_pool(name="epool", bufs=8))
        dpool = ctx.enter_context(tc.tile_pool(name="dpool", bufs=3))
        opool = ctx.enter_context(tc.tile_pool(name="opool", bufs=4))
        # explicit PSUM partitioning (8 banks total): 2 long-lived AV
        # accumulators (throttles attention to 2 blocks in flight), 2 for
        # score groups, 2 for projection groups, 2 shared by the
        # denominator-broadcast and out-projection churn; out-proj also
        # borrows psj (free during attention windows) for 4-bank pipelining
        pso = ctx.enter_context(tc.tile_pool(name="pso", bufs=2, space="PSUM"))
        pss = ctx.enter_context(tc.tile_pool(name="pss", bufs=2, space="PSUM"))
        psj = ctx.enter_context(tc.tile_pool(name="psj", bufs=2, space="PSUM"))
        psm = ctx.enter_context(tc.tile_pool(name="psm", bufs=2, space="PSUM"))

        # --- constants -----------------------------------------------------
        # ones first: it gates the PE warm-up burst below
        ones_bf = const.tile([128, 128], CDT, tag="ones", name="ones")
        nc.gpsimd.memset(ones_bf[:], 1.0)

        # PE warm-up: ~2us of dummy matmuls on the ones tile while the first
        # DMAs are still in flight, so the HAM clock gate is already open
        # when real work arrives
        warm = psm.tile([128, 512], F32, tag="m", name="warm")
        for _ in range(16):
            nc.tensor.matmul(warm[:, 0:128], ones_bf[:], ones_bf[:],
                             start=True, stop=True, skip_group_check=True)
        wdump = const.tile([128, 128], F32, tag="wdump", name="wdump")
        nc.scalar.copy(wdump[:], warm[:, 0:128])

        # Diagonal causal mask: key row x attends query col y iff y - x >= 0.
        trimask = const.tile([128, 128], CDT, tag="trimask", name="trimask")
        nc.gpsimd.memset(trimask[:], 1.0)
        nc.gpsimd.affine_select(
            out=trimask[:], in_=trimask[:], compare_op=mybir.AluOpType.is_ge,
            fill=0.0, base=0, pattern=[[1, 128]], channel_multiplier=-1)
        ebias = const.tile([128, 1], F32, tag="ebias", name="ebias")
        nc.gpsimd.memset(ebias[:], EXP_BIAS)

        # --- weights (wq and the first x chunk first: they gate the first
        # matmul; the rest can stream in behind) -----------------------------
        # first QT group needs wq chunk k and xt chunk k together: issue the
        # pieces interleaved on sync in 4-chunk granules so the k-loop can
        # start as soon as the first granule lands.  wk streams in 4-chunk
        # granules on the scalar ring in parallel (needed one group later),
        # wv after it; wo is NOT issued here -- it isn't consumed until the
        # first attention window (~100us in) and its 2MB would steal HBM
        # bandwidth from the startup-critical wq/x/wk stream
        wq_s = wpool.tile([128, ndch * ML], CDT, tag="wq", name="wq_s")
        wk_s = wpool.tile([128, ndch * ML], CDT, tag="wk", name="wk_s")
        xt0 = xpool.tile([128, ndch * 512], CDT, tag="xt", name="xt")
        w4 = ndch // 4
        for piece in range(4):
            k0 = piece * w4
            nc.sync.dma_start(
                wq_s[:, k0 * ML:(k0 + w4) * ML],
                wqT_e[:, k0 * ML:(k0 + w4) * ML])
            nc.sync.dma_start(
                xt0[:, k0 * 512:(k0 + w4) * 512],
                xT_e[:, k0 * 512:(k0 + w4) * 512])
            nc.scalar.dma_start(
                wk_s[:, k0 * ML:(k0 + w4) * ML],
                wkT_e[:, k0 * ML:(k0 + w4) * ML])

        wv_s = wpool.tile([128, ndch * ML], CDT, tag="wv", name="wv")
        nc.scalar.dma_start(wv_s[:], wvT_e[:])
        wo_s = []

        def load_wo():
            for h in range(HPC):
                t = wpool.tile([128, d], CDT, tag=f"wo{h}", name=f"wo{h}")
                nc.scalar.dma_start(t[:], woT_e[h * 128:(h + 1) * 128, :])
                wo_s.append(t)

        # --- persistent per-batch activations ------------------------------
        QT = [[qkv.tile([128, s], CDT, tag=f"qt{bb}{h}", name=f"qt{bb}{h}") for h in range(HPC)]
              for bb in range(b)]
        KT = [[qkv.tile([128, s], CDT, tag=f"kt{bb}{h}", name=f"kt{bb}{h}") for h in range(HPC)]
              for bb in range(b)]
        V = [qkv.tile([128, nblk * ML], CDT, tag=f"v{bb}", name=f"v{bb}") for bb in range(b)]
        YT = [[qkv.tile([128, s], CDT, tag=f"yt{bb}{h}", name=f"yt{bb}{h}") for h in range(HPC)]
              for bb in range(b)]

        xt_tiles = {(0, 0): xt0}

        def load_xt(bb):
            # emit the x-chunk DMAs early in the sync stream so they are
            # never head-of-line blocked behind output DMAs that wait on
            # attention results
            for tt in range(ttch):
                if (bb, tt) in xt_tiles:
                    continue
                it = bb * ttch + tt
                t = xpool.tile([128, ndch * 512], CDT, tag="xt", name="xt")
                nc.sync.dma_start(
                    t[:], xT_e[:, it * ndch * 512:(it + 1) * ndch * 512])
                xt_tiles[(bb, tt)] = t

        def qk_group(bb, tt, m2, wsb, dst):
            xt = xt_tiles[(bb, tt)]
            pst = psj.tile([128, 512], F32, tag="j", name="pst")
            for k in range(ndch):
                nc.tensor.matmul(
                    pst[:],
                    wsb[:, k * ML + m2 * 128: k * ML + m2 * 128 + 128],
                    xt[:, k * 512:(k + 1) * 512],
                    start=(k == 0), stop=(k == ndch - 1))
            # alternate the psum->sbuf cast between DVE and ACT so woven
            # projection groups don't pile onto the attention-loaded DVE
            if m2 % 2 == 0:
                nc.vector.tensor_copy(
                    dst[bb][m2][:, tt * 512:(tt + 1) * 512], pst[:])
            else:
                nc.scalar.copy(
                    dst[bb][m2][:, tt * 512:(tt + 1) * 512], pst[:])

        def v_group(bb, tt, v4):
            xt = xt_tiles[(bb, tt)]
            pst = psj.tile([128, 512], F32, tag="j", name="pst")
            for k in range(ndch):
                nc.tensor.matmul(
                    pst[:, 0:ML],
                    xt[:, k * 512 + v4 * 128: k * 512 + v4 * 128 + 128],
                    wv_s[:, k * ML:(k + 1) * ML],
                    start=(k == 0), stop=(k == ndch - 1))
            blk = tt * 4 + v4
            nc.scalar.copy(V[bb][:, blk * ML:(blk + 1) * ML], pst[:, 0:ML])

        def proj_units(bb):
            # phase 1 as a list of independently emittable 16-matmul units so
            # the next batch's projection can be woven between attention
            # blocks of the current one (the per-engine schedule is
            # priority-ordered: stalls can only be filled by work that is
            # EMITTED inside the stall region)
            units = []
            for tt in range(ttch):
                for m2 in range(HPC):
                    for wsb, dst in ((wq_s, QT), (wk_s, KT)):
                        units.append(
                            lambda bb=bb, tt=tt, m2=m2, wsb=wsb, dst=dst:
                            qk_group(bb, tt, m2, wsb, dst))
                for v4 in range(4):
                    units.append(
                        lambda bb=bb, tt=tt, v4=v4: v_group(bb, tt, v4))
            return units

        op_ctr = [0]

        def outproj_block(bb, n):
            # --- phase 3: partial out-projection for one 128-token block ---
            # fine-grained staging (one tile + DMA per 512-wide e chunk) so
            # the output stream drains smoothly in the background; fp16
            # staging halves the HBM write traffic
            t0 = n * 128
            for e2 in range((nech + 1) // 2):
                # (128,1024) staging: two psum tiles, one copy on ACT and one
                # on DVE, a single coarser output DMA (halves the sync
                # engine's serial DMA-issue load).  ps_p alternates between
                # the psm and psj pools: 4 banks of pipelining so the copy
                # latency never stalls the matmul stream (psj is idle during
                # attention windows, which is when out-proj runs)
                npair = min(2, nech - e2 * 2)
                ost = opool.tile([128, 1024], CDT, tag="ost", name="ost")
                for sub in range(npair):
                    ec = e2 * 2 + sub
                    pool = psm if op_ctr[0] % 2 == 0 else psj
                    op_ctr[0] += 1
                    ps_p = pool.tile([128, 512], F32,
                                     tag="m" if pool is psm else "j",
                                     name="ps_p")
                    for h in range(HPC):
                        nc.tensor.matmul(
                            ps_p[:],
                            YT[bb][h][:, t0:t0 + 128],
                            wo_s[h][:, ec * 512:(ec + 1) * 512],
                            start=(h == 0), stop=(h == HPC - 1))
                    if sub == 0:
                        nc.scalar.copy(ost[:, 0:512], ps_p[:])
                    else:
                        nc.vector.tensor_copy(ost[:, 512:1024], ps_p[:])
                nc.sync.dma_start(
                    out_e[bb * s + t0: bb * s + t0 + 128,
                          e2 * 1024:e2 * 1024 + npair * 512],
                    ost[:, 0:npair * 512])

        def drain(fillers, k):
            for _ in range(min(k, len(fillers))):
                fillers.pop(0)()

        def attn_block(bb, qj, fillers=None, defer_to=None):
            # --- phase 2: causal attention for one 512-query block ---------
            # softmax denominator via two fp16 DVE accumulator chains (2x DVE
            # mode; exp values <= ~9.4 and <= 16 summands per element keep
            # fp16 accumulation within ~2e-3 relative)
            for h in range(HPC):
                nki = 4 * qj + 4
                q0 = qj * 512
                ps_o = pso.tile([128, 512], F32, tag="o", name="ps_o")
                # two independent DVE accumulator chains halve the
                # serial-add latency per block
                acc = dpool.tile([128, 512], CDT, tag="acc", name="acc")
                acc1 = dpool.tile([128, 512], CDT, tag="acc1", name="acc1")
                for ki in range(nki):
                    # diagonal key chunks only see queries >= their own
                    # position: narrow to columns [c0:512)
                    g = ki - 4 * qj
                    c0 = 128 * g if g > 0 else 0
                    ps_s = pss.tile([128, 512], F32, tag="s", name="ps_s")
                    nc.tensor.matmul(
                        ps_s[:, c0:512],
                        KT[bb][h][:, ki * 128:(ki + 1) * 128],
                        QT[bb][h][:, q0 + c0:q0 + 512],
                        start=True, stop=True)
                    et = epool.tile([128, 512], CDT, tag="et", name="et")
                    nc.scalar.activation(
                        et[:, c0:512], ps_s[:, c0:512],
                        mybir.ActivationFunctionType.Exp, scale=SCALE,
                        bias=ebias[:, 0:1])
                    if g >= 0:
                        # diagonal-block causal mask on the otherwise-idle
                        # GPSIMD engine (SBUF-only operands), keeping DVE
                        # off this part of the exp->AV critical chain
                        nc.gpsimd.tensor_mul(
                            et[:, c0:c0 + 128], et[:, c0:c0 + 128],
                            trimask[:])
                    eng = nc.vector
                    if ki < 2:
                        dst = acc if ki == 0 else acc1
                        if c0 > 0:
                            eng.memset(dst[:, 0:c0], 0.0)
                        eng.tensor_copy(dst[:, c0:512], et[:, c0:512])
                    else:
                        dst = acc if ki % 2 == 0 else acc1
                        eng.tensor_add(dst[:, c0:512], dst[:, c0:512],
                                       et[:, c0:512])
                    nc.tensor.matmul(
                        ps_o[:, c0:512],
                        V[bb][:, ki * ML + h * 128: ki * ML + h * 128 + 128],
                        et[:, c0:512],
                        start=(ki == 0), stop=(ki == nki - 1),
                        skip_group_check=True)
                    if fillers and ki % 3 == 2:
                        drain(fillers, 1)
                rbc = dpool.tile([128, 512], F32, tag="rbc", name="rbc")
                # one ones(128,128) matmul both sums the 128 partitions
                # of acc and broadcasts the denominator across all 128
                # output partitions.
                accb = dpool.tile([128, 512], CDT, tag="accb", name="accb")
                nc.vector.tensor_tensor(accb[:], acc[:], acc1[:],
                                        mybir.AluOpType.add)
                ps_r = psm.tile([128, 512], F32, tag="m", name="ps_r")
                nc.tensor.matmul(ps_r[:], ones_bf[:],
                                 accb[:], start=True, stop=True)
                nc.vector.reciprocal_approx_fast(out=rbc[:], in_=ps_r[:])
                nc.vector.tensor_mul(YT[bb][h][:, q0:q0 + 512],
                                     ps_o[:], rbc[:])
            for n in range(4 * qj, 4 * qj + 4):
                if defer_to is not None:
                    defer_to.append(lambda bb=bb, n=n: outproj_block(bb, n))
                else:
                    if fillers:
                        drain(fillers, 1)
                    outproj_block(bb, n)

        # per-batch stagger: the next batch's projection units are emitted
        # INSIDE the current batch's attention blocks (the per-engine
        # schedule is priority-ordered, so softmax-latency stalls can only
        # be filled by work emitted within them).  qj descending: longest
        # attention blocks first.
        deferred = []
        for bb in range(b):
            load_xt(bb)
            if bb == 0:
                for u in proj_units(0):
                    u()
                load_wo()
            # fillers for this batch's attention window: the previous
            # batch's deferred out-projection plus the next batch's
            # projection units
            fillers = list(deferred)
            deferred = []
            if bb + 1 < b:
                load_xt(bb + 1)
                fillers += proj_units(bb + 1)
                defer_to = deferred   # push own out-proj into next window
            else:
                defer_to = None
            if bb + 1 < b:
                order = list(range(nqj - 1, -1, -1))
            else:
                # last batch: qj0 first (ready after the first projection
                # chunk, warms the softmax chain early), then longest-first
                order = [0] + list(range(nqj - 1, 0, -1))
            for qj in order:
                attn_block(bb, qj, fillers, defer_to)
            for u in fillers:
                u()

    nc.compile()
    return nc


_NC_CACHE = {}


def _get_nc():
    if "nc" not in _NC_CACHE:
        _NC_CACHE["nc"] = build_nc()
    return _NC_CACHE["nc"]


def shard_inputs(x, wq, wk, wv, wo):
    """Host-side sharding: 2 heads (256 out dims) per core; fp16 cast.

    All inputs are packed into the exact on-chip tile layouts so every DMA
    reads long contiguous runs per partition:
      xT  -> (128, b*ttch*ndch*512): tile (bb,tt) at flat index it holds
             [p, it, k, tau] = x[bb, tt*512+tau, k*128+p]
      w?T -> (128, ndch*ML): [p, k, m] = w[core_slice(m), k*128+p]
    """
    ndch = D // 128
    ttch = S // 512
    xf = np.asarray(x, dtype=np.float32).reshape(B, ttch, 512, ndch, 128)
    # [bb, tt, tau, k, p] -> [p, bb, tt, k, tau]
    xT = np.ascontiguousarray(xf.transpose(4, 0, 1, 3, 2)).reshape(
        128, B * ttch * ndch * 512).astype(NPCDT)

    def packw(w, sl):
        # w[sl] is (ML, D); want [p, k, m] = w[sl][m, k*128+p]
        a = np.asarray(w, dtype=np.float32)[sl].reshape(ML, ndch, 128)
        return np.ascontiguousarray(a.transpose(2, 1, 0)).reshape(
            128, ndch * ML).astype(NPCDT)

    in_maps = []
    for c in range(N_CORES):
        sl = slice(c * ML, (c + 1) * ML)
        in_maps.append({
            "xT": xT,
            "wqT": packw(wq, sl),
            "wkT": packw(wk, sl),
            "wvT": packw(wv, sl),
            "woT": np.ascontiguousarray(np.asarray(wo)[:, sl].T).astype(NPCDT),
        })
    return in_maps


def run(inputs, trace=False, trace_cores=None):
    nc = _get_nc()
    in_maps = shard_inputs(inputs["x"], inputs["wq"], inputs["wk"],
                           inputs["wv"], inputs["wo"])
    res = run_bass_kernel_spmd(nc, in_maps, core_ids=list(range(N_CORES)),
                               trace=trace, trace_cores=trace_cores)
    out = res.results[0]["out"].astype(np.float32)
    for c in range(1, N_CORES):
        out = out + res.results[c]["out"].astype(np.float32)
    return out.reshape(B, S, D), res


def kernel(**inputs) -> np.ndarray:
    out, _ = run(inputs, trace=False)
    return out


# revision 10
# speedup vs baseline: 1.1640x; 1.0131x over previous
"""Trainium2 Bass kernel: causal multi-head attention, tensor-parallel over heads.

Problem: x:(2,2048,2048) f32, wq/wk/wv/wo:(2048,2048) f32 (torch Linear layout,
applied as x @ W.T).  out = MHA_causal(x) @ wo.T, 16 heads x 128 dim.

Sharding: 2 heads per core (8 cores).  Each core computes Q/K/V projections for
its 2 heads, causal attention, and a partial out-projection through its slice
of wo; the host sums the 8 fp16 partial outputs (the all-reduce equivalent).

Per-core layouts (all compute in fp16, accumulation in f32 PSUM):
  xT   (D, NTOK)   d-major      : streaming rhs for Q^T/K^T proj, stationary for V
  Q^T  (dh, tok)   per (b,h)    : moving operand of scores
  K^T  (dh, tok)   per (b,h)    : stationary operand of scores
  V    (tok, m)    token-major  : stationary operand of AV
  s_T  (key, q)    scores psum  : softmax denominator via DVE fp16 chains
  y^T  (dh, tok)   normalized   : stationary operand of out-proj

All HBM->SBUF inputs are pre-packed on the host into the exact on-chip tile
layout so each DMA moves 8-16KB contiguous runs per partition (full rate);
the naive (D, ntok) gather runs at ~78GB/s due to 1KB descriptors.
"""

import sys

for _p in ("/opt/trn_rl_repo", "/root/.axon_site/_ro/trn_rl_repo"):
    if _p not in sys.path:
        sys.path.append(_p)

from contextlib import ExitStack

import ml_dtypes
import numpy as np

import concourse.bass as bass
import concourse.bacc as bacc
import concourse.mybir as mybir
import concourse.tile as tile
from concourse.bass_utils import run_bass_kernel_spmd

# compute dtype: fp16 (same TensorE speed as bf16, 2x DVE mode, and 3 more
# mantissa bits); PSUM accumulation is always f32
CDT = mybir.dt.float16
F32 = mybir.dt.float32
NPCDT = np.float16
# exp(x - 4*ln2) = exp(x)/16 keeps fp16 softmax denominators well inside range;
# the scale cancels exactly in the normalization
EXP_BIAS = -4.0 * float(np.log(2.0))

N_CORES = 8
B, S, D = 2, 2048, 2048
NH, DH = 16, 128
HPC = NH // N_CORES          # heads per core
ML = HPC * DH                # local head dims per core (256)
SCALE = 1.0 / float(np.sqrt(DH))


def build_nc(b=B, s=S, d=D):
    """Build the per-core Bass graph.  Parameterized so a scaled-down variant
    can run under CoreSim; the shipped kernel always uses the defaults."""
    ntok = b * s
    ndch = d // 128       # contraction chunks for projections
    nech = d // 512       # 512-wide out-proj e chunks
    ttch = s // 512       # 512-token chunks per batch
    nqj = s // 512        # 512-query chunks per (b, h)
    nblk = s // 128       # 128-token blocks per batch

    nc = bacc.Bacc("TRN2", target_bir_lowering=False, debug=False,
                   num_devices=N_CORES)
    # host-tiled inputs: xT as per-(bb,tt) tiles of (128, ndch*512) laid out
    # contiguously; w{q,k,v} as (128, ndch*ML) in (p, k, m) order
    xT_e = nc.dram_tensor("xT", [128, b * ttch * ndch * 512], CDT,
                          kind="ExternalInput").ap()
    wqT_e = nc.dram_tensor("wqT", [128, ndch * ML], CDT,
                           kind="ExternalInput").ap()
    wkT_e = nc.dram_tensor("wkT", [128, ndch * ML], CDT,
                           kind="ExternalInput").ap()
    wvT_e = nc.dram_tensor("wvT", [128, ndch * ML], CDT,
                           kind="ExternalInput").ap()
    woT_e = nc.dram_tensor("woT", [ML, d], CDT, kind="ExternalInput").ap()
    out_e = nc.dram_tensor("out", [ntok, d], CDT, kind="ExternalOutput").ap()

    with tile.TileContext(nc) as tc, ExitStack() as ctx:
        const = ctx.enter_context(tc.tile_pool(name="const", bufs=1))
        wpool = ctx.enter_context(tc.tile_pool(name="wpool", bufs=1))
        xpool = ctx.enter_context(tc.tile_pool(name="xpool", bufs=4))
        qkv = ctx.enter_context(tc.tile_pool(name="qkv", bufs=1))
        epool = ctx.enter_context(tc.tile_pool(name="epool", bufs=8))
        dpool = ctx.enter_context(tc.tile_pool(name="dpool", bufs=3))
        opool = ctx.enter_context(tc.tile_pool(name="opool", bufs=4))
        # explicit PSUM partitioning (8 banks total): 2 long-lived AV
        # accumulators (throttles attention to 2 blocks in flight), 2 for
        # score groups, 2 for projection groups, 2 shared by the
        # denominator-broadcast and out-projection churn; out-proj also
        # borrows psj (free during attention windows) for 4-bank pipelining
        pso = ctx.enter_context(tc.tile_pool(name="pso", bufs=2, space="PSUM"))
        pss = ctx.enter_context(tc.tile_pool(name="pss", bufs=2, space="PSUM"))
        psj = ctx.enter_context(tc.tile_pool(name="psj", bufs=2, space="PSUM"))
        psm = ctx.enter_context(tc.tile_pool(name="psm", bufs=2, space="PSUM"))

        # --- constants -----------------------------------------------------
        # ones first: it gates the PE warm-up burst below
        ones_bf = const.tile([128, 128], CDT, tag="ones", name="ones")
        nc.gpsimd.memset(ones_bf[:], 1.0)

        # PE warm-up: ~2us of dummy matmuls on the ones tile while the first
        # DMAs are still in flight, so the HAM clock gate is already open
        # when real work arrives
        warm = psm.tile([128, 512], F32, tag="m", name="warm")
        for _ in range(16):
            nc.tensor.matmul(warm[:, 0:128], ones_bf[:], ones_bf[:],
                             start=True, stop=True, skip_group_check=True)
        wdump = const.tile([128, 128], F32, tag="wdump", name="wdump")
        nc.scalar.copy(wdump[:], warm[:, 0:128])

        # Diagonal causal mask: key row x attends query col y iff y - x >= 0.
        trimask = const.tile([128, 128], CDT, tag="trimask", name="trimask")
        nc.gpsimd.memset(trimask[:], 1.0)
        nc.gpsimd.affine_select(
            out=trimask[:], in_=trimask[:], compare_op=mybir.AluOpType.is_ge,
            fill=0.0, base=0, pattern=[[1, 128]], channel_multiplier=-1)
        ebias = const.tile([128, 1], F32, tag="ebias", name="ebias")
        nc.gpsimd.memset(ebias[:], EXP_BIAS)

        # --- weights (wq and the first x chunk first: they gate the first
        # matmul; the rest can stream in behind) -----------------------------
        # first QT group needs wq chunk k and xt chunk k together: issue the
        # pieces interleaved on sync in 4-chunk granules so the k-loop can
        # start as soon as the first granule lands.  wk streams in 4-chunk
        # granules on the scalar ring in parallel (needed one group later),
        # wv after it; wo is NOT issued here -- it isn't consumed until the
        # first attention window (~100us in) and its 2MB would steal HBM
        # bandwidth from the startup-critical wq/x/wk stream
        wq_s = wpool.tile([128, ndch * ML], CDT, tag="wq", name="wq_s")
        wk_s = wpool.tile([128, ndch * ML], CDT, tag="wk", name="wk_s")
        xt0 = xpool.tile([128, ndch * 512], CDT, tag="xt", name="xt")
        w8 = ndch // 8
        for piece in range(8):
            k0 = piece * w8
            nc.sync.dma_start(
                wq_s[:, k0 * ML:(k0 + w8) * ML],
                wqT_e[:, k0 * ML:(k0 + w8) * ML])
            nc.sync.dma_start(
                xt0[:, k0 * 512:(k0 + w8) * 512],
                xT_e[:, k0 * 512:(k0 + w8) * 512])
            nc.scalar.dma_start(
                wk_s[:, k0 * ML:(k0 + w8) * ML],
                wkT_e[:, k0 * ML:(k0 + w8) * ML])

        wv_s = wpool.tile([128, ndch * ML], CDT, tag="wv", name="wv")
        nc.scalar.dma_start(wv_s[:], wvT_e[:])
        wo_s = []

        def load_wo():
            for h in range(HPC):
                t = wpool.tile([128, d], CDT, tag=f"wo{h}", name=f"wo{h}")
                nc.scalar.dma_start(t[:], woT_e[h * 128:(h + 1) * 128, :])
                wo_s.append(t)

        # --- persistent per-batch activations ------------------------------
        QT = [[qkv.tile([128, s], CDT, tag=f"qt{bb}{h}", name=f"qt{bb}{h}") for h in range(HPC)]
              for bb in range(b)]
        KT = [[qkv.tile([128, s], CDT, tag=f"kt{bb}{h}", name=f"kt{bb}{h}") for h in range(HPC)]
              for bb in range(b)]
        V = [qkv.tile([128, nblk * ML], CDT, tag=f"v{bb}", name=f"v{bb}") for bb in range(b)]
        YT = [[qkv.tile([128, s], CDT, tag=f"yt{bb}{h}", name=f"yt{bb}{h}") for h in range(HPC)]
              for bb in range(b)]

        xt_tiles = {(0, 0): xt0}

        def load_xt(bb):
            # emit the x-chunk DMAs early in the sync stream so they are
            # never head-of-line blocked behind output DMAs that wait on
            # attention results
            for tt in range(ttch):
                if (bb, tt) in xt_tiles:
                    continue
                it = bb * ttch + tt
                t = xpool.tile([128, ndch * 512], CDT, tag="xt", name="xt")
                nc.sync.dma_start(
                    t[:], xT_e[:, it * ndch * 512:(it + 1) * ndch * 512])
                xt_tiles[(bb, tt)] = t

        def qk_group(bb, tt, m2, wsb, dst):
            xt = xt_tiles[(bb, tt)]
            pst = psj.tile([128, 512], F32, tag="j", name="pst")
            for k in range(ndch):
                nc.tensor.matmul(
                    pst[:],
                    wsb[:, k * ML + m2 * 128: k * ML + m2 * 128 + 128],
                    xt[:, k * 512:(k + 1) * 512],
                    start=(k == 0), stop=(k == ndch - 1))
            # alternate the psum->sbuf cast between DVE and ACT so woven
            # projection groups don't pile onto the attention-loaded DVE
            if m2 % 2 == 0:
                nc.vector.tensor_copy(
                    dst[bb][m2][:, tt * 512:(tt + 1) * 512], pst[:])
            else:
                nc.scalar.copy(
                    dst[bb][m2][:, tt * 512:(tt + 1) * 512], pst[:])

        def v_group(bb, tt, v4):
            xt = xt_tiles[(bb, tt)]
            pst = psj.tile([128, 512], F32, tag="j", name="pst")
            for k in range(ndch):
                nc.tensor.matmul(
                    pst[:, 0:ML],
                    xt[:, k * 512 + v4 * 128: k * 512 + v4 * 128 + 128],
                    wv_s[:, k * ML:(k + 1) * ML],
                    start=(k == 0), stop=(k == ndch - 1))
            blk = tt * 4 + v4
            nc.scalar.copy(V[bb][:, blk * ML:(blk + 1) * ML], pst[:, 0:ML])

        def proj_units(bb):
            # phase 1 as a list of independently emittable 16-matmul units so
            # the next batch's projection can be woven between attention
            # blocks of the current one (the per-engine schedule is
            # priority-ordered: stalls can only be filled by work that is
            # EMITTED inside the stall region)
            units = []
            for tt in range(ttch):
                # wq groups before wk groups: at startup wk streams in on the
                # scalar ring one group-time behind wq
                for wsb, dst in ((wq_s, QT), (wk_s, KT)):
                    for m2 in range(HPC):
                        units.append(
                            lambda bb=bb, tt=tt, m2=m2, wsb=wsb, dst=dst:
                            qk_group(bb, tt, m2, wsb, dst))
                for v4 in range(4):
                    units.append(
                        lambda bb=bb, tt=tt, v4=v4: v_group(bb, tt, v4))
            return units

        op_ctr = [0]

        def outproj_block(bb, n):
            # --- phase 3: partial out-projection for one 128-token block ---
            # fine-grained staging (one tile + DMA per 512-wide e chunk) so
            # the output stream drains smoothly in the background; fp16
            # staging halves the HBM write traffic
            t0 = n * 128
            for e2 in range((nech + 1) // 2):
                # (128,1024) staging: two psum tiles, one copy on ACT and one
                # on DVE, a single coarser output DMA (halves the sync
                # engine's serial DMA-issue load).  ps_p alternates between
                # the psm and psj pools: 4 banks of pipelining so the copy
                # latency never stalls the matmul stream (psj is idle during
                # attention windows, which is when out-proj runs)
                npair = min(2, nech - e2 * 2)
                ost = opool.tile([128, 1024], CDT, tag="ost", name="ost")
                for sub in range(npair):
                    ec = e2 * 2 + sub
                    pool = psm if op_ctr[0] % 2 == 0 else psj
                    op_ctr[0] += 1
                    ps_p = pool.tile([128, 512], F32,
                                     tag="m" if pool is psm else "j",
                                     name="ps_p")
                    for h in range(HPC):
                        nc.tensor.matmul(
                            ps_p[:],
                            YT[bb][h][:, t0:t0 + 128],
                            wo_s[h][:, ec * 512:(ec + 1) * 512],
                            start=(h == 0), stop=(h == HPC - 1))
                    if sub == 0:
                        nc.scalar.copy(ost[:, 0:512], ps_p[:])
                    else:
                        nc.vector.tensor_copy(ost[:, 512:1024], ps_p[:])
                nc.sync.dma_start(
                    out_e[bb * s + t0: bb * s + t0 + 128,
                          e2 * 1024:e2 * 1024 + npair * 512],
                    ost[:, 0:npair * 512])

        def drain(fillers, k):
            for _ in range(min(k, len(fillers))):
                fillers.pop(0)()

        def attn_block(bb, qj, fillers=None, defer_to=None, last=False):
            # --- phase 2: causal attention for one 512-query block ---------
            # softmax denominator via two fp16 DVE accumulator chains (2x DVE
            # mode; exp values <= ~9.4 and <= 16 summands per element keep
            # fp16 accumulation within ~2e-3 relative)
            pending = []

            def finalize(bb, h, q0, ps_o, acc, acc1, chunks):
                # one ones(128,128) matmul both sums the 128 partitions of
                # acc and broadcasts the denominator across all 128 output
                # partitions.  The ones-matmul depends on the DVE chain, so
                # callers defer emitting this behind fresh independent MMs
                # (the PE queue is in-order: a waiting matmul blocks it).
                # chunks>1 emits the chain in column chunks so dependent
                # out-proj blocks can start before the whole 512-query
                # normalization finishes (used to shorten the kernel tail).
                accb = dpool.tile([128, 512], CDT, tag="accb", name="accb")
                rbc = dpool.tile([128, 512], F32, tag="rbc", name="rbc")
                ps_r = psm.tile([128, 512], F32, tag="m", name="ps_r")
                cw = 512 // chunks
                for c in range(chunks):
                    cs = slice(c * cw, (c + 1) * cw)
                    nc.vector.tensor_tensor(accb[:, cs], acc[:, cs],
                                            acc1[:, cs], mybir.AluOpType.add)
                    nc.tensor.matmul(ps_r[:, cs], ones_bf[:], accb[:, cs],
                                     start=True, stop=True)
                    nc.vector.reciprocal_approx_fast(out=rbc[:, cs],
                                                     in_=ps_r[:, cs])
                    nc.vector.tensor_mul(YT[bb][h][:, q0 + c * cw:
                                                   q0 + (c + 1) * cw],
                                         ps_o[:, cs], rbc[:, cs])
                    yield

            for h in range(HPC):
                nki = 4 * qj + 4
                q0 = qj * 512
                ps_o = pso.tile([128, 512], F32, tag="o", name="ps_o")
                # two independent DVE accumulator chains halve the
                # serial-add latency per block
                acc = dpool.tile([128, 512], CDT, tag="acc", name="acc")
                acc1 = dpool.tile([128, 512], CDT, tag="acc1", name="acc1")
                for ki in range(nki):
                    # diagonal key chunks only see queries >= their own
                    # position: narrow to columns [c0:512)
                    g = ki - 4 * qj
                    c0 = 128 * g if g > 0 else 0
                    ps_s = pss.tile([128, 512], F32, tag="s", name="ps_s")
                    nc.tensor.matmul(
                        ps_s[:, c0:512],
                        KT[bb][h][:, ki * 128:(ki + 1) * 128],
                        QT[bb][h][:, q0 + c0:q0 + 512],
                        start=True, stop=True)
                    et = epool.tile([128, 512], CDT, tag="et", name="et")
                    nc.scalar.activation(
                        et[:, c0:512], ps_s[:, c0:512],
                        mybir.ActivationFunctionType.Exp, scale=SCALE,
                        bias=ebias[:, 0:1])
                    if g >= 0:
                        # diagonal-block causal mask on the otherwise-idle
                        # GPSIMD engine (SBUF-only operands), keeping DVE
                        # off this part of the exp->AV critical chain
                        nc.gpsimd.tensor_mul(
                            et[:, c0:c0 + 128], et[:, c0:c0 + 128],
                            trimask[:])
                    eng = nc.vector
                    if ki < 2:
                        dst = acc if ki == 0 else acc1
                        if c0 > 0:
                            eng.memset(dst[:, 0:c0], 0.0)
                        eng.tensor_copy(dst[:, c0:512], et[:, c0:512])
                    else:
                        dst = acc if ki % 2 == 0 else acc1
                        eng.tensor_add(dst[:, c0:512], dst[:, c0:512],
                                       et[:, c0:512])
                    nc.tensor.matmul(
                        ps_o[:, c0:512],
                        V[bb][:, ki * ML + h * 128: ki * ML + h * 128 + 128],
                        et[:, c0:512],
                        start=(ki == 0), stop=(ki == nki - 1),
                        skip_group_check=True)
                    if ki == 1 and pending:
                        # emit the previous sub-block's finalize here, behind
                        # two fresh kis of independent matmuls
                        for _ in pending.pop(0):
                            pass
                    if fillers and ki % 3 == 2:
                        drain(fillers, 1)
                if not (last and h == HPC - 1):
                    pending.append(finalize(bb, h, q0, ps_o, acc, acc1, 1))
            if last:
                # tail: chunk the last sub-block's finalize and interleave
                # its out-proj blocks so the output pipeline starts early
                for gen in pending:
                    for _ in gen:
                        pass
                fin = finalize(bb, HPC - 1, qj * 512, ps_o, acc, acc1, 4)
                for n in range(4 * qj, 4 * qj + 4):
                    next(fin, None)
                    outproj_block(bb, n)
                for _ in fin:
                    pass
                return
            for gen in pending:
                for _ in gen:
                    pass
            for n in range(4 * qj, 4 * qj + 4):
                if defer_to is not None:
                    defer_to.append(lambda bb=bb, n=n: outproj_block(bb, n))
                else:
                    if fillers:
                        drain(fillers, 1)
                    outproj_block(bb, n)

        # per-batch stagger: the next batch's projection units are emitted
        # INSIDE the current batch's attention blocks (the per-engine
        # schedule is priority-ordered, so softmax-latency stalls can only
        # be filled by work emitted within them).  qj descending: longest
        # attention blocks first.
        deferred = []
        for bb in range(b):
            load_xt(bb)
            if bb == 0:
                for u in proj_units(0):
                    u()
                load_wo()
            # fillers for this batch's attention window: the previous
            # batch's deferred out-projection plus the next batch's
            # projection units
            fillers = list(deferred)
            deferred = []
            if bb + 1 < b:
                load_xt(bb + 1)
                fillers += proj_units(bb + 1)
                defer_to = deferred   # push own out-proj into next window
            else:
                defer_to = None
            if bb + 1 < b:
                order = list(range(nqj - 1, -1, -1))
            else:
                # last batch: qj0 first (ready after the first projection
                # chunk, warms the softmax chain early), then longest-first
                order = [0] + list(range(nqj - 1, 0, -1))
            for idx, qj in enumerate(order):
                attn_block(bb, qj, fillers, defer_to,
                           last=(bb + 1 == b and idx + 1 == len(order)))
            for u in fillers:
                u()

    nc.compile()
    return nc


_NC_CACHE = {}


def _get_nc():
    if "nc" not in _NC_CACHE:
        _NC_CACHE["nc"] = build_nc()
    return _NC_CACHE["nc"]


def shard_inputs(x, wq, wk, wv, wo):
    """Host-side sharding: 2 heads (256 out dims) per core; fp16 cast.

    All inputs are packed into the exact on-chip tile layouts so every DMA
    reads long contiguous runs per partition:
      xT  -> (128, b*ttch*ndch*512): tile (bb,tt) at flat index it holds
             [p, it, k, tau] = x[bb, tt*512+tau, k*128+p]
      w?T -> (128, ndch*ML): [p, k, m] = w[core_slice(m), k*128+p]
    """
    ndch = D // 128
    ttch = S // 512
    xf = np.asarray(x, dtype=np.float32).reshape(B, ttch, 512, ndch, 128)
    # [bb, tt, tau, k, p] -> [p, bb, tt, k, tau]
    xT = np.ascontiguousarray(xf.transpose(4, 0, 1, 3, 2)).reshape(
        128, B * ttch * ndch * 512).astype(NPCDT)

    def packw(w, sl):
        # w[sl] is (ML, D); want [p, k, m] = w[sl][m, k*128+p]
        a = np.asarray(w, dtype=np.float32)[sl].reshape(ML, ndch, 128)
        return np.ascontiguousarray(a.transpose(2, 1, 0)).reshape(
            128, ndch * ML).astype(NPCDT)

    in_maps = []
    for c in range(N_CORES):
        sl = slice(c * ML, (c + 1) * ML)
        in_maps.append({
            "xT": xT,
            "wqT": packw(wq, sl),
            "wkT": packw(wk, sl),
            "wvT": packw(wv, sl),
            "woT": np.ascontiguousarray(np.asarray(wo)[:, sl].T).astype(NPCDT),
        })
    return in_maps


def run(inputs, trace=False, trace_cores=None):
    nc = _get_nc()
    in_maps = shard_inputs(inputs["x"], inputs["wq"], inputs["wk"],
                           inputs["wv"], inputs["wo"])
    res = run_bass_kernel_spmd(nc, in_maps, core_ids=list(range(N_CORES)),
                               trace=trace, trace_cores=trace_cores)
    out = res.results[0]["out"].astype(np.float32)
    for c in range(1, N_CORES):
        out = out + res.results[c]["out"].astype(np.float32)
    return out.reshape(B, S, D), res


def kernel(**inputs) -> np.ndarray:
    out, _ = run(inputs, trace=False)
    return out
